# revision 1
# baseline (speedup 1.0000x reference)
"""Trainium2 Bass kernel for nn_Block_77318001263203 (dense transformer block).

Distribution over 8 NeuronCores: data-parallel over batch (2 groups of 4
cores) x tensor-parallel over heads (4 heads/core) for attention+proj,
4-way-chunked ReduceScatter of the proj partials over each 4-core group
(pipelined with attention compute; each chunk hands every rank one
128-token block, so rank r owns the strided token set
{512*ck + 128*r + j}), then token-parallel FFN with full (replicated)
FFN weights — no second collective. All matmuls run as float32r (full PE
rate, ~2e-4 rel err).

kernel(**inputs) takes the FULL inputs from setup_inputs() and returns the
FULL [2, 2048, 1024] output.
"""

import numpy as np

import concourse.bass as bass
import concourse.mybir as mybir
import concourse.tile as tile
from concourse import bacc
from concourse.bass_utils import run_bass_kernel_spmd
from concourse.masks import make_identity

# problem dims (hardcoded per the harness contract)
B, S, D = 2, 2048, 1024
H, HS, F = 16, 64, 4096
EPS = 1e-5
P = 128
NCORES = 8
TP = 4  # cores per batch group
HPC = H // TP  # heads per core = 4
SL = S // TP  # tokens owned per core = 512 (4 strided blocks of 128)
QT = 512  # query tile
KB = 128  # key block
NCK = 4  # reduce-scatter chunks
NEG = -1.0e9  # additive causal mask (exp underflows to exactly 0)

f32 = mybir.dt.float32
f32r = mybir.dt.float32r

REPLICA_GROUPS = [[0, 1, 2, 3], [4, 5, 6, 7]]


def _bcast_row_ap(t, row, width):
    """DMA-source AP broadcasting row `row` of DRAM tensor t to 128 partitions."""
    return bass.AP(tensor=t, offset=row * width, ap=[[0, P], [1, width]])


def build_bass():
    nc = bacc.Bacc("TRN2", target_bir_lowering=False, debug=False, num_devices=NCORES)

    xT = nc.dram_tensor("xT", [D, S], f32, kind="ExternalInput").ap()
    xs = nc.dram_tensor("xs", [SL, D], f32, kind="ExternalInput").ap()
    wq2 = nc.dram_tensor("wq2", [D, HPC * HS], f32, kind="ExternalInput").ap()
    wk2 = nc.dram_tensor("wk2", [D, HPC * HS], f32, kind="ExternalInput").ap()
    wv4 = nc.dram_tensor("wv4", [D, HPC * HS], f32, kind="ExternalInput").ap()
    wp = nc.dram_tensor("wp", [HPC * HS, D], f32, kind="ExternalInput").ap()
    w1 = nc.dram_tensor("w1", [D, F], f32, kind="ExternalInput").ap()
    w2 = nc.dram_tensor("w2", [F, D], f32, kind="ExternalInput").ap()
    cvec = nc.dram_tensor("cvec", [6, D], f32, kind="ExternalInput").ap()
    b1d = nc.dram_tensor("b1d", [F], f32, kind="ExternalInput").ap()
    out = nc.dram_tensor("out", [SL, D], f32, kind="ExternalOutput").ap()

    # per-chunk collective bounce buffers (separate tensors -> precise deps)
    rs_in = [nc.dram_tensor(f"rs_in{c}", [S // NCK, D], f32) for c in range(NCK)]
    rs_out = [nc.dram_tensor(f"rs_out{c}", [P, D], f32) for c in range(NCK)]

    # additive causal triangle mask for the 128x128 diagonal block:
    # keep (0) where t <= q, NEG where t > q
    m_np = np.where(
        np.arange(KB)[:, None] <= np.arange(KB)[None, :], 0.0, NEG
    ).astype(np.float32)
    masks_dram = nc.inline_tensor(m_np, name="causal_mask")

    with tile.TileContext(nc) as tc:
        with tc.tile_pool(name="const", bufs=1) as constp:
            ident_f = constp.tile([P, P], f32)
            make_identity(nc, ident_f)
            ident = constp.tile([P, P], f32r)
            nc.vector.tensor_copy(ident, ident_f)
            eps_t = constp.tile([P, 1], f32)
            nc.vector.memset(eps_t, EPS)
            b1_sb = constp.tile([P, F // P], f32)
            nc.sync.dma_start(b1_sb, b1d.rearrange("(ko p) -> p ko", p=P))
            g2b = constp.tile([P, D], f32)
            nc.gpsimd.dma_start(g2b, _bcast_row_ap(cvec.tensor, 3, D))
            be2b = constp.tile([P, D], f32)
            nc.gpsimd.dma_start(be2b, _bcast_row_ap(cvec.tensor, 4, D))
            b2b = constp.tile([P, D], f32)
            nc.gpsimd.dma_start(b2b, _bcast_row_ap(cvec.tensor, 5, D))

            # ---------------- Phase A: QKV + attention + proj + chunked RS ---
            with (
                tc.tile_pool(name="wqkvp", bufs=1) as wqkvp,
                tc.tile_pool(name="qkvo", bufs=1) as qkvo,
                tc.tile_pool(name="xrp", bufs=2) as xrp,
                tc.tile_pool(name="smallp", bufs=4) as smallp,
                tc.tile_pool(name="projp", bufs=3) as projp,
            ):
                # QKV projection weights first (on the critical path)
                wq_sb = wqkvp.tile([P, D // P, HPC * HS], f32r, tag="wq")
                nc.sync.dma_start(
                    wq_sb, wq2.rearrange("(ko p) m -> p ko m", p=P).bitcast(f32r)
                )
                wk_sb = wqkvp.tile([P, D // P, HPC * HS], f32r, tag="wk")
                nc.sync.dma_start(
                    wk_sb, wk2.rearrange("(ko p) m -> p ko m", p=P).bitcast(f32r)
                )
                wv_sb = wqkvp.tile([P, D // P, HPC * HS], f32r, tag="wv")
                nc.sync.dma_start(
                    wv_sb, wv4.rearrange("(ko p) m -> p ko m", p=P).bitcast(f32r)
                )

                # outputs of QKV: qT/kT per head pair, v (+ones col) per head
                q2T = qkvo.tile([P, 2, S], f32r, tag="q2T")
                k2T = qkvo.tile([P, 2, S], f32r, tag="k2T")
                v4e = qkvo.tile([P, S // P, HPC * (HS + 1)], f32r, tag="v4e")
                attnT = qkvo.tile([P, 2, S], f32r, tag="attnT")
                ones4 = qkvo.tile([P, HPC, 1], f32, tag="ones4")
                nc.vector.memset(ones4, 1.0)

                with tc.tile_pool(name="ps_qkv", bufs=4, space="PSUM") as psq:
                    for tt in range(S // QT):
                        xr = xrp.tile([P, D // P, QT], f32r, tag="xr")
                        nc.sync.dma_start(
                            xr,
                            xT[:, tt * QT : (tt + 1) * QT]
                            .rearrange("(ko p) m -> p ko m", p=P)
                            .bitcast(f32r),
                        )
                        for hp in range(2):
                            qps = psq.tile([P, QT], f32, tag="qk")
                            for ko in range(D // P):
                                nc.tensor.matmul(
                                    qps,
                                    wq_sb[:, ko, hp * P : (hp + 1) * P],
                                    xr[:, ko, :],
                                    start=(ko == 0),
                                    stop=(ko == D // P - 1),
                                )
                            nc.vector.tensor_copy(
                                q2T[:, hp, tt * QT : (tt + 1) * QT], qps
                            )
                            kps = psq.tile([P, QT], f32, tag="qk")
                            for ko in range(D // P):
                                nc.tensor.matmul(
                                    kps,
                                    wk_sb[:, ko, hp * P : (hp + 1) * P],
                                    xr[:, ko, :],
                                    start=(ko == 0),
                                    stop=(ko == D // P - 1),
                                )
                            nc.vector.tensor_copy(
                                k2T[:, hp, tt * QT : (tt + 1) * QT], kps
                            )
                        for mt in range(QT // P):
                            vps = psq.tile([P, HPC * HS], f32, tag="v")
                            for ko in range(D // P):
                                nc.tensor.matmul(
                                    vps,
                                    xr[:, ko, mt * P : (mt + 1) * P],
                                    wv_sb[:, ko, :],
                                    start=(ko == 0),
                                    stop=(ko == D // P - 1),
                                )
                            idx = tt * (QT // P) + mt
                            vv = v4e[:, idx, :].rearrange("p (h e) -> p h e", e=HS + 1)
                            nc.vector.tensor_copy(
                                vv[:, :, 0:HS],
                                vps.rearrange("p (h e) -> p h e", e=HS),
                            )
                            nc.vector.tensor_copy(vv[:, :, HS : HS + 1], ones4)

                # proj weights + masks: needed later, keep off the startup path
                wp_sb = wqkvp.tile([P, (HPC * HS) // P, D], f32r, tag="wp")
                nc.sync.dma_start(
                    wp_sb, wp.rearrange("(ko p) n -> p ko n", p=P).bitcast(f32r)
                )
                masks_sb = wqkvp.tile([P, KB], f32, tag="masks")
                nc.sync.dma_start(masks_sb, masks_dram.ap())

                with (
                    tc.tile_pool(name="ps_sc", bufs=2, space="PSUM") as pssc,
                    tc.tile_pool(name="ps_at", bufs=3, space="PSUM") as psat,
                    tc.tile_pool(name="ps_pr", bufs=1, space="PSUM") as pspr,
                ):
                    for qt in range(S // QT):
                        nkb = 4 * qt + 4
                        qsl = slice(qt * QT, (qt + 1) * QT)
                        for hp in range(2):
                            apair = psat.tile([HS + 1, QT], f32, tag="at")
                            apodd = psat.tile([HS + 1, QT], f32, tag="at")
                            for kb in range(nkb):
                                ksl = slice(kb * KB, (kb + 1) * KB)
                                moff = kb - 4 * qt
                                # diagonal blocks: columns [0, KB*moff) are fully
                                # masked -> skip them entirely
                                q0 = KB * moff if moff > 0 else 0
                                qr = slice(qt * QT + q0, (qt + 1) * QT)
                                sp = pssc.tile([P, 2, QT], f32, tag="sc")
                                nc.tensor.matmul(
                                    sp[:, 0, q0:],
                                    k2T[0:HS, hp, ksl],
                                    q2T[0:HS, hp, qr],
                                    start=True,
                                    stop=True,
                                    tile_position=(0, 0),
                                )
                                nc.tensor.matmul(
                                    sp[:, 1, q0:],
                                    k2T[HS : 2 * HS, hp, ksl],
                                    q2T[HS : 2 * HS, hp, qr],
                                    start=True,
                                    stop=True,
                                    tile_position=(64, 0),
                                )
                                if moff >= 0:
                                    dia = slice(KB * moff, KB * (moff + 1))
                                    nc.vector.tensor_add(
                                        sp[:, :, dia],
                                        sp[:, :, dia],
                                        masks_sb[:, None, :].to_broadcast(
                                            (P, 2, KB)
                                        ),
                                    )
                                ee = smallp.tile([P, 2, QT], f32r, tag="ee")
                                nc.scalar.activation(
                                    out=ee[:, :, q0:],
                                    in_=sp[:, :, q0:],
                                    func=mybir.ActivationFunctionType.Exp,
                                    scale=float(HS) ** -0.5,
                                )
                                he = (2 * hp) * (HS + 1)
                                ho = (2 * hp + 1) * (HS + 1)
                                nc.tensor.matmul(
                                    apair[:, q0:],
                                    v4e[:, kb, he : he + HS + 1],
                                    ee[:, 0, q0:],
                                    start=(kb == 0),
                                    stop=(kb == nkb - 1),
                                )
                                nc.tensor.matmul(
                                    apodd[:, q0:],
                                    v4e[:, kb, ho : ho + HS + 1],
                                    ee[:, 1, q0:],
                                    start=(kb == 0),
                                    stop=(kb == nkb - 1),
                                )
                            # quick PSUM->SBUF copy (frees accumulators), then
                            # normalize in SBUF off the PE critical path
                            for par, aps in ((0, apair), (1, apodd)):
                                ua = smallp.tile([HS + 1, QT], f32, tag="ua")
                                nc.vector.tensor_copy(ua, aps)
                                rec = smallp.tile([1, QT], f32, tag="rec")
                                nc.vector.reciprocal(rec, ua[HS : HS + 1, :])
                                bc = smallp.tile([HS, QT], f32, tag="bc")
                                nc.gpsimd.partition_broadcast(bc, rec)
                                nc.vector.tensor_mul(
                                    attnT[par * HS : (par + 1) * HS, hp, qsl],
                                    ua[0:HS, :],
                                    bc,
                                )
                        # proj for this qt's 4 token tiles, then RS chunk qt
                        for mtl in range(4):
                            mt = 4 * qt + mtl
                            prj = projp.tile([P, D], f32, tag="prj")
                            for nh in range(D // QT):
                                pps = pspr.tile([P, QT], f32, tag="pr")
                                for ko in range(2):
                                    nc.tensor.matmul(
                                        pps,
                                        attnT[:, ko, mt * P : (mt + 1) * P],
                                        wp_sb[:, ko, nh * QT : (nh + 1) * QT],
                                        start=(ko == 0),
                                        stop=(ko == 1),
                                    )
                                nc.vector.tensor_copy(
                                    prj[:, nh * QT : (nh + 1) * QT], pps
                                )
                            nc.sync.dma_start(
                                rs_in[qt].ap()[mtl * P : (mtl + 1) * P, :], prj
                            )
                        nc.gpsimd.collective_compute(
                            "ReduceScatter",
                            mybir.AluOpType.add,
                            replica_groups=REPLICA_GROUPS,
                            ins=[rs_in[qt].ap().opt()],
                            outs=[rs_out[qt].ap().opt()],
                        )

            # ---------------- Phase B: LN1 + FFN + LN2 ----------------
            with tc.tile_pool(name="ffn_keep", bufs=1) as keep:
                x1r = keep.tile([P, SL // P, D], f32r, tag="x1r")
                hT = keep.tile([P, F // P, SL], f32r, tag="hT")
                x1tp_cm = tc.tile_pool(name="x1tp", bufs=1)
                x1tp = x1tp_cm.__enter__()
                x1T = x1tp.tile([P, D // P, SL], f32r, tag="x1T")

                with (
                    tc.tile_pool(name="ln1p", bufs=2) as ln1p,
                    tc.tile_pool(name="ln1c", bufs=1) as ln1c,
                    tc.tile_pool(name="ps_tr", bufs=2, space="PSUM") as pstr,
                ):
                    g1b = ln1c.tile([P, D], f32, tag="g1b")
                    nc.gpsimd.dma_start(g1b, _bcast_row_ap(cvec.tensor, 1, D))
                    be1b = ln1c.tile([P, D], f32, tag="be1b")
                    nc.gpsimd.dma_start(be1b, _bcast_row_ap(cvec.tensor, 2, D))
                    bpb = ln1c.tile([P, D], f32, tag="bpb")
                    nc.gpsimd.dma_start(bpb, _bcast_row_ap(cvec.tensor, 0, D))

                    for st in range(SL // P):
                        y = ln1p.tile([P, D], f32, tag="y")
                        nc.sync.dma_start(y, rs_out[st].ap())
                        xst = ln1p.tile([P, D], f32, tag="xst")
                        nc.sync.dma_start(xst, xs[st * P : (st + 1) * P, :])
                        nc.vector.tensor_add(y, y, xst)
                        nc.vector.tensor_add(y, y, bpb)
                        stats = ln1p.tile([P, 2, 6], f32, tag="stats")
                        yv = y.rearrange("p (s d) -> p s d", s=2)
                        nc.vector.bn_stats(out=stats[:, 0, :], in_=yv[:, 0, :])
                        nc.vector.bn_stats(out=stats[:, 1, :], in_=yv[:, 1, :])
                        mv = ln1p.tile([P, 2], f32, tag="mv")
                        nc.vector.bn_aggr(out=mv, in_=stats)
                        rstd = ln1p.tile([P, 1], f32, tag="rstd")
                        nc.scalar.activation(
                            out=rstd,
                            in_=mv[:, 1:2],
                            func=mybir.ActivationFunctionType.Sqrt,
                            bias=eps_t,
                            scale=1.0,
                        )
                        nc.vector.reciprocal(rstd, rstd)
                        tmp = ln1p.tile([P, D], f32, tag="tmp")
                        nc.vector.tensor_scalar(
                            out=tmp,
                            in0=y,
                            scalar1=mv[:, 0:1],
                            scalar2=rstd,
                            op0=mybir.AluOpType.subtract,
                            op1=mybir.AluOpType.mult,
                        )
                        nc.vector.tensor_mul(tmp, tmp, g1b)
                        nc.vector.tensor_add(x1r[:, st, :], tmp, be1b)
                        # transpose this token tile into x1T
                        for dk in range(D // P):
                            tp = pstr.tile([P, P], f32r, tag="tp")
                            nc.tensor.transpose(
                                tp, x1r[:, st, dk * P : (dk + 1) * P], ident
                            )
                            nc.vector.tensor_copy(
                                x1T[:, dk, st * P : (st + 1) * P], tp
                            )

                # FFN first matmul: hT[f, tok] = w1.T @ x1T, relu(+b1) fused
                with (
                    tc.tile_pool(name="w1p", bufs=3) as w1p,
                    tc.tile_pool(name="ps_h", bufs=2, space="PSUM") as psh,
                ):
                    for ft in range(F // P):
                        w1t = w1p.tile([P, D // P, P], f32r, tag="w1t")
                        nc.sync.dma_start(
                            w1t,
                            w1[:, ft * P : (ft + 1) * P]
                            .rearrange("(ko p) m -> p ko m", p=P)
                            .bitcast(f32r),
                        )
                        hps = psh.tile([P, SL], f32, tag="h")
                        for ko in range(D // P):
                            nc.tensor.matmul(
                                hps,
                                w1t[:, ko, :],
                                x1T[:, ko, :],
                                start=(ko == 0),
                                stop=(ko == D // P - 1),
                            )
                        nc.scalar.activation(
                            out=hT[:, ft, :],
                            in_=hps,
                            func=mybir.ActivationFunctionType.Relu,
                            bias=b1_sb[:, ft : ft + 1],
                            scale=1.0,
                        )
                x1tp_cm.__exit__(None, None, None)

                # FFN second matmul (directly in [tok, d] layout) + residual + LN2
                with (
                    tc.tile_pool(name="w2p", bufs=2) as w2p,
                    tc.tile_pool(name="zp", bufs=1) as zp,
                    tc.tile_pool(name="ln2p", bufs=2) as ln2p,
                    tc.tile_pool(name="ps_y", bufs=4, space="PSUM") as psy,
                ):
                    NQ = 512  # d-half width
                    NKO = F // (2 * P)  # 16 k-subtiles per w2 tile
                    z = zp.tile([P, SL // P, D], f32, tag="z")
                    for dtq in range(D // NQ):
                        ypss = [
                            psy.tile([P, NQ], f32, tag="yq", name=f"yq_{dtq}_{i}")
                            for i in range(SL // P)
                        ]
                        for kh in range(2):
                            w2t = w2p.tile([P, NKO, NQ], f32r, tag="w2t")
                            nc.sync.dma_start(
                                w2t,
                                w2[
                                    kh * (F // 2) : (kh + 1) * (F // 2),
                                    dtq * NQ : (dtq + 1) * NQ,
                                ]
                                .rearrange("(ko p) n -> p ko n", p=P)
                                .bitcast(f32r),
                            )
                            for mt in range(SL // P):
                                for ko in range(NKO):
                                    nc.tensor.matmul(
                                        ypss[mt],
                                        hT[:, kh * NKO + ko, mt * P : (mt + 1) * P],
                                        w2t[:, ko, :],
                                        start=(kh == 0 and ko == 0),
                                        stop=(kh == 1 and ko == NKO - 1),
                                    )
                        dsl = slice(dtq * NQ, (dtq + 1) * NQ)
                        for mt in range(SL // P):
                            nc.vector.tensor_add(
                                z[:, mt, dsl], ypss[mt], x1r[:, mt, dsl]
                            )
                    # z += b2, then LN2 -> out
                    for mt in range(SL // P):
                        zm = z[:, mt, :]
                        nc.vector.tensor_add(zm, zm, b2b)
                        stats = ln2p.tile([P, 2, 6], f32, tag="stats2")
                        zv = zm.rearrange("p (s d) -> p s d", s=2)
                        nc.vector.bn_stats(out=stats[:, 0, :], in_=zv[:, 0, :])
                        nc.vector.bn_stats(out=stats[:, 1, :], in_=zv[:, 1, :])
                        mv = ln2p.tile([P, 2], f32, tag="mv2")
                        nc.vector.bn_aggr(out=mv, in_=stats)
                        rstd = ln2p.tile([P, 1], f32, tag="rstd2")
                        nc.scalar.activation(
                            out=rstd,
                            in_=mv[:, 1:2],
                            func=mybir.ActivationFunctionType.Sqrt,
                            bias=eps_t,
                            scale=1.0,
                        )
                        nc.vector.reciprocal(rstd, rstd)
                        o = ln2p.tile([P, D], f32, tag="o")
                        nc.vector.tensor_scalar(
                            out=o,
                            in0=zm,
                            scalar1=mv[:, 0:1],
                            scalar2=rstd,
                            op0=mybir.AluOpType.subtract,
                            op1=mybir.AluOpType.mult,
                        )
                        nc.vector.tensor_mul(o, o, g2b)
                        nc.vector.tensor_add(o, o, be2b)
                        nc.sync.dma_start(out[mt * P : (mt + 1) * P, :], o)

    nc.compile()
    return nc


_NC_CACHE = []


def _get_nc():
    if not _NC_CACHE:
        _NC_CACHE.append(build_bass())
    return _NC_CACHE[0]


def _token_blocks(r):
    """Global token rows (within a batch element) owned by rank r, as NCK
    blocks of 128: block ck covers rows [512*ck + 128*r, 512*ck + 128*r + 128)."""
    return [slice(QT * ck + P * r, QT * ck + P * r + P) for ck in range(NCK)]


def make_in_maps(x, wq, wk, wv, w_proj, b_proj, w1, b1, w2, b2, g1, be1, g2, be2):
    x = np.asarray(x, dtype=np.float32)
    cat = lambda w, h0: np.ascontiguousarray(
        np.concatenate(
            [np.asarray(w[h0 + i], dtype=np.float32) for i in range(HPC)], axis=1
        )
    )
    cvec_rows = [b_proj, g1, be1, g2, be2, b2]
    cvec = np.ascontiguousarray(
        np.stack([np.asarray(v, dtype=np.float32) for v in cvec_rows])
    )
    w1c = np.ascontiguousarray(np.asarray(w1, dtype=np.float32))
    w2c = np.ascontiguousarray(np.asarray(w2, dtype=np.float32))
    b1c = np.ascontiguousarray(np.asarray(b1, dtype=np.float32))
    wpc = np.ascontiguousarray(np.asarray(w_proj, dtype=np.float32))
    xTs = [np.ascontiguousarray(x[g].T) for g in range(B)]
    in_maps = []
    for c in range(NCORES):
        g, r = divmod(c, TP)
        h0 = HPC * r
        xs_blocks = np.concatenate([x[g, blk] for blk in _token_blocks(r)], axis=0)
        in_maps.append(
            {
                "xT": xTs[g],
                "xs": np.ascontiguousarray(xs_blocks),
                "wq2": cat(wq, h0),
                "wk2": cat(wk, h0),
                "wv4": cat(wv, h0),
                "wp": np.ascontiguousarray(wpc[HPC * HS * r : HPC * HS * (r + 1)]),
                "w1": w1c,
                "w2": w2c,
                "cvec": cvec,
                "b1d": b1c,
            }
        )
    return in_maps


def assemble(results):
    full = np.empty((B, S, D), dtype=np.float32)
    for c in range(NCORES):
        g, r = divmod(c, TP)
        o = results[c]["out"]
        for ck, blk in enumerate(_token_blocks(r)):
            full[g, blk] = o[ck * P : (ck + 1) * P]
    return full


def kernel(**inputs):
    nc = _get_nc()
    in_maps = make_in_maps(**inputs)
    res = run_bass_kernel_spmd(nc, in_maps, core_ids=list(range(NCORES)))
    return assemble(res.results)



# revision 14
# speedup vs baseline: 1.0510x; 1.0510x over previous
"""Trainium2 Bass kernel for nn_Block_77318001263203 (dense transformer block).

Distribution over 8 NeuronCores: data-parallel over batch (2 groups of 4
cores) x tensor-parallel over heads (4 heads/core) for attention+proj,
4-way-chunked ReduceScatter of the proj partials over each 4-core group
(pipelined with attention compute; each chunk hands every rank one
128-token block, so rank r owns the strided token set
{512*ck + 128*r + j}), then token-parallel FFN with full (replicated)
FFN weights — no second collective.

v2 performance notes (vs the f32r v1 at ~630us device time):
- all matmul operands in bf16 (weights host-cast, activations cast on the
  PSUM->SBUF drain): halves LDWEIGHTS time, SBUF footprint and DMA bytes;
  PSUM accumulation stays f32.
- ReduceScatter payloads in bf16 (half the wire bytes).
- softmax denominators: batched reciprocal_approx_fast on stacked rows
  instead of 16 single-partition `reciprocal` calls (~60us of DVE time).
- LN1 work per 128-token chunk is interleaved into the attention loop so
  it runs as soon as that chunk's ReduceScatter lands.
- FFN w1 stays SBUF-resident (bf16), w2 streams per 128-row tile; the w1
  load and the final RS chunk overlap in the shadow of the attention tail.
- DMA traffic split across the sync/scalar/gpsimd queues so bulk weight
  loads never head-of-line-block the proj->rs_in stores.

kernel(**inputs) takes the FULL inputs from setup_inputs() and returns the
FULL [2, 2048, 1024] float32 output.
"""

import numpy as np
import ml_dtypes

import concourse.bass as bass
import concourse.mybir as mybir
import concourse.tile as tile
from concourse import bacc
from concourse.bass_utils import run_bass_kernel_spmd
from concourse.masks import make_identity

# problem dims (hardcoded per the harness contract)
B, S, D = 2, 2048, 1024
H, HS, F = 16, 64, 4096
EPS = 1e-5
P = 128
NCORES = 8
TP = 4  # cores per batch group
HPC = H // TP  # heads per core = 4
SL = S // TP  # tokens owned per core = 512 (4 strided blocks of 128)
QT = 512  # query tile
KB = 128  # key block
NCK = 4  # reduce-scatter chunks
NEG = -1.0e9  # additive causal mask (exp underflows to exactly 0)

f32 = mybir.dt.float32
bf16 = mybir.dt.bfloat16
np_bf16 = ml_dtypes.bfloat16

REPLICA_GROUPS = [[0, 1, 2, 3], [4, 5, 6, 7]]

DEBUG = False  # adds intermediate-dump outputs (dbg_*) for bisection


def _bcast_row_ap(t, row, width):
    """DMA-source AP broadcasting row `row` of DRAM tensor t to 128 partitions."""
    return bass.AP(tensor=t, offset=row * width, ap=[[0, P], [1, width]])


def build_bass():
    nc = bacc.Bacc("TRN2", target_bir_lowering=False, debug=False, num_devices=NCORES)

    xT = nc.dram_tensor("xT", [D, S], bf16, kind="ExternalInput").ap()
    xs = nc.dram_tensor("xs", [SL, D], f32, kind="ExternalInput").ap()
    wq2 = nc.dram_tensor("wq2", [D, HPC * HS], bf16, kind="ExternalInput").ap()
    wk2 = nc.dram_tensor("wk2", [D, HPC * HS], bf16, kind="ExternalInput").ap()
    wv4 = nc.dram_tensor("wv4", [D, HPC * HS], bf16, kind="ExternalInput").ap()
    wp = nc.dram_tensor("wp", [HPC * HS, D], bf16, kind="ExternalInput").ap()
    w1 = nc.dram_tensor("w1", [D, F], bf16, kind="ExternalInput").ap()
    w2 = nc.dram_tensor("w2", [F, D], bf16, kind="ExternalInput").ap()
    cvec = nc.dram_tensor("cvec", [6, D], f32, kind="ExternalInput").ap()
    b1d = nc.dram_tensor("b1d", [F], f32, kind="ExternalInput").ap()
    out = nc.dram_tensor("out", [SL, D], f32, kind="ExternalOutput").ap()
    if DEBUG:
        dbg_q2T = nc.dram_tensor("dbg_q2T", [P, 2, S], bf16, kind="ExternalOutput").ap()
        dbg_k2T = nc.dram_tensor("dbg_k2T", [P, 2, S], bf16, kind="ExternalOutput").ap()
        dbg_v4e = nc.dram_tensor(
            "dbg_v4e", [P, S // P, HPC * (HS + 1)], bf16, kind="ExternalOutput"
        ).ap()
        dbg_attnT = nc.dram_tensor(
            "dbg_attnT", [P, 2, S], bf16, kind="ExternalOutput"
        ).ap()
        dbg_rsout = nc.dram_tensor(
            "dbg_rsout", [NCK, P, D], bf16, kind="ExternalOutput"
        ).ap()
        dbg_x1r = nc.dram_tensor(
            "dbg_x1r", [P, SL // P, D], f32, kind="ExternalOutput"
        ).ap()
        dbg_hT = nc.dram_tensor(
            "dbg_hT", [P, F // P, SL], bf16, kind="ExternalOutput"
        ).ap()

    # per-chunk collective bounce buffers (separate tensors -> precise deps)
    rs_in = [nc.dram_tensor(f"rs_in{c}", [S // NCK, D], bf16) for c in range(NCK)]
    rs_out = [nc.dram_tensor(f"rs_out{c}", [P, D], bf16) for c in range(NCK)]

    # additive causal triangle mask for the 128x128 diagonal block:
    # keep (0) where t <= q, NEG where t > q
    m_np = np.where(
        np.arange(KB)[:, None] <= np.arange(KB)[None, :], 0.0, NEG
    ).astype(np.float32)
    masks_dram = nc.inline_tensor(m_np, name="causal_mask")

    with tile.TileContext(nc) as tc:
        with tc.tile_pool(name="const", bufs=1) as constp:
            ident_f = constp.tile([P, P], f32)
            make_identity(nc, ident_f)
            identb = constp.tile([P, P], bf16)
            nc.vector.tensor_copy(identb, ident_f)
            eps_t = constp.tile([P, 1], f32)
            nc.vector.memset(eps_t, EPS)
            b1_sb = constp.tile([P, F // P], f32)
            nc.gpsimd.dma_start(b1_sb, b1d.rearrange("(ko p) -> p ko", p=P))
            # residual source rows for this core (gpsimd queue: off the
            # critical sync-queue path)
            xs_all = constp.tile([P, SL // P, D], f32)
            nc.gpsimd.dma_start(xs_all, xs.rearrange("(s p) d -> p s d", p=P))

            # LN1 constants (needed from mid-attention on)
            bpb = constp.tile([P, D], f32)
            nc.gpsimd.dma_start(bpb, _bcast_row_ap(cvec.tensor, 0, D))
            g1b = constp.tile([P, D], f32)
            nc.gpsimd.dma_start(g1b, _bcast_row_ap(cvec.tensor, 1, D))
            be1b = constp.tile([P, D], f32)
            nc.gpsimd.dma_start(be1b, _bcast_row_ap(cvec.tensor, 2, D))

            # keep-tiles that cross the attention/FFN phase boundary
            ffn_keep_cm = tc.tile_pool(name="ffn_keep", bufs=1)
            keep = ffn_keep_cm.__enter__()
            x1r = keep.tile([P, SL // P, D], f32, tag="x1r")
            x1b = keep.tile([P, SL // P, D], bf16, tag="x1b")

            # ------------- Phase A: QKV -------------
            with tc.tile_pool(name="qkvo", bufs=1) as qkvo:
                q2T = qkvo.tile([P, 2, S], bf16, tag="q2T")
                k2T = qkvo.tile([P, 2, S], bf16, tag="k2T")
                v4e = qkvo.tile([P, S // P, HPC * (HS + 1)], bf16, tag="v4e")
                attnT = qkvo.tile([P, 2, S], bf16, tag="attnT")
                # ones columns of v4e: set everything to 1, data overwrites
                nc.gpsimd.memset(v4e, 1.0)

                with (
                    tc.tile_pool(name="wqkvp", bufs=1) as wqkvp,
                    tc.tile_pool(name="xrp", bufs=2) as xrp,
                    tc.tile_pool(name="ps_qkv", bufs=4, space="PSUM") as psq,
                ):
                    wq_sb = wqkvp.tile([P, D // P, HPC * HS], bf16, tag="wq")
                    nc.sync.dma_start(wq_sb, wq2.rearrange("(ko p) m -> p ko m", p=P))
                    wk_sb = wqkvp.tile([P, D // P, HPC * HS], bf16, tag="wk")
                    nc.sync.dma_start(wk_sb, wk2.rearrange("(ko p) m -> p ko m", p=P))
                    wv_sb = wqkvp.tile([P, D // P, HPC * HS], bf16, tag="wv")
                    nc.sync.dma_start(wv_sb, wv4.rearrange("(ko p) m -> p ko m", p=P))

                    for tt in range(S // QT):
                        xr = xrp.tile([P, D // P, QT], bf16, tag="xr")
                        nc.sync.dma_start(
                            xr,
                            xT[:, tt * QT : (tt + 1) * QT].rearrange(
                                "(ko p) m -> p ko m", p=P
                            ),
                        )
                        for hp in range(2):
                            qps = psq.tile([P, QT], f32, tag="qk")
                            for ko in range(D // P):
                                nc.tensor.matmul(
                                    qps,
                                    wq_sb[:, ko, hp * P : (hp + 1) * P],
                                    xr[:, ko, :],
                                    start=(ko == 0),
                                    stop=(ko == D // P - 1),
                                )
                            nc.vector.tensor_copy(
                                q2T[:, hp, tt * QT : (tt + 1) * QT], qps
                            )
                            kps = psq.tile([P, QT], f32, tag="qk")
                            for ko in range(D // P):
                                nc.tensor.matmul(
                                    kps,
                                    wk_sb[:, ko, hp * P : (hp + 1) * P],
                                    xr[:, ko, :],
                                    start=(ko == 0),
                                    stop=(ko == D // P - 1),
                                )
                            nc.vector.tensor_copy(
                                k2T[:, hp, tt * QT : (tt + 1) * QT], kps
                            )
                        for mt in range(QT // P):
                            vps = psq.tile([P, HPC * HS], f32, tag="v")
                            for ko in range(D // P):
                                nc.tensor.matmul(
                                    vps,
                                    xr[:, ko, mt * P : (mt + 1) * P],
                                    wv_sb[:, ko, :],
                                    start=(ko == 0),
                                    stop=(ko == D // P - 1),
                                )
                            idx = tt * (QT // P) + mt
                            vv = v4e[:, idx, :].rearrange("p (h e) -> p h e", e=HS + 1)
                            nc.scalar.copy(
                                vv[:, :, 0:HS],
                                vps.rearrange("p (h e) -> p h e", e=HS),
                            )

                # ------------- attention + proj + chunked RS + LN1 -------------
                with tc.tile_pool(name="wpm", bufs=1) as wpm:
                    wp_sb = wpm.tile([P, (HPC * HS) // P, D], bf16, tag="wp")
                    nc.sync.dma_start(wp_sb, wp.rearrange("(ko p) n -> p ko n", p=P))
                    masks_sb = wpm.tile([P, KB], f32, tag="masks")
                    nc.sync.dma_start(masks_sb, masks_dram.ap())

                    def emit_ln1(st, ln1p):
                        """LN1 for token block st (runs once rs_out[st] landed)."""
                        y_bf = ln1p.tile([P, D], bf16, tag="ybf", name=f"ybf{st}")
                        nc.gpsimd.dma_start(y_bf, rs_out[st].ap())
                        yf = ln1p.tile([P, D], f32, tag="yf", name=f"yf{st}")
                        nc.scalar.copy(yf, y_bf)
                        nc.vector.tensor_add(yf, yf, xs_all[:, st, :])
                        nc.vector.tensor_add(yf, yf, bpb)
                        stats = ln1p.tile([P, 2, 6], f32, tag="st", name=f"st{st}")
                        yv = yf.rearrange("p (s d) -> p s d", s=2)
                        nc.vector.bn_stats(out=stats[:, 0, :], in_=yv[:, 0, :])
                        nc.vector.bn_stats(out=stats[:, 1, :], in_=yv[:, 1, :])
                        mv = ln1p.tile([P, 2], f32, tag="mv", name=f"mv{st}")
                        nc.vector.bn_aggr(out=mv, in_=stats)
                        rstd = ln1p.tile([P, 1], f32, tag="rs", name=f"rs{st}")
                        nc.scalar.activation(
                            out=rstd,
                            in_=mv[:, 1:2],
                            func=mybir.ActivationFunctionType.Sqrt,
                            bias=eps_t,
                            scale=1.0,
                        )
                        nc.vector.reciprocal(rstd, rstd)
                        tmp = ln1p.tile([P, D], f32, tag="tmp", name=f"tmp{st}")
                        nc.vector.tensor_scalar(
                            out=tmp,
                            in0=yf,
                            scalar1=mv[:, 0:1],
                            scalar2=rstd,
                            op0=mybir.AluOpType.subtract,
                            op1=mybir.AluOpType.mult,
                        )
                        nc.vector.tensor_mul(tmp, tmp, g1b)
                        nc.vector.tensor_add(x1r[:, st, :], tmp, be1b)
                        nc.vector.tensor_copy(x1b[:, st, :], x1r[:, st, :])

                    with (
                        tc.tile_pool(name="ln1p", bufs=2) as ln1p,
                        tc.tile_pool(name="smallp", bufs=4) as smallp,
                        tc.tile_pool(name="denp", bufs=2) as denp,
                        tc.tile_pool(name="projp", bufs=3) as projp,
                        tc.tile_pool(name="ps_sc", bufs=2, space="PSUM") as pssc,
                        tc.tile_pool(name="ps_at", bufs=3, space="PSUM") as psat,
                        tc.tile_pool(name="ps_pr", bufs=1, space="PSUM") as pspr,
                    ):
                        for qt in range(S // QT):
                            nkb = 4 * qt + 4
                            qsl = slice(qt * QT, (qt + 1) * QT)
                            for hp in range(2):
                                apair = psat.tile([HS + 1, QT], f32, tag="at")
                                apodd = psat.tile([HS + 1, QT], f32, tag="at")
                                for kb in range(nkb):
                                    ksl = slice(kb * KB, (kb + 1) * KB)
                                    moff = kb - 4 * qt
                                    # diagonal blocks: columns [0, KB*moff) are
                                    # fully masked -> skip them entirely
                                    q0 = KB * moff if moff > 0 else 0
                                    qr = slice(qt * QT + q0, (qt + 1) * QT)
                                    sp = pssc.tile([P, 2, QT], f32, tag="sc")
                                    nc.tensor.matmul(
                                        sp[:, 0, q0:],
                                        k2T[0:HS, hp, ksl],
                                        q2T[0:HS, hp, qr],
                                        start=True,
                                        stop=True,
                                        tile_position=(0, 0),
                                    )
                                    nc.tensor.matmul(
                                        sp[:, 1, q0:],
                                        k2T[HS : 2 * HS, hp, ksl],
                                        q2T[HS : 2 * HS, hp, qr],
                                        start=True,
                                        stop=True,
                                        tile_position=(64, 0),
                                    )
                                    if moff >= 0:
                                        dia = slice(KB * moff, KB * (moff + 1))
                                        nc.vector.tensor_add(
                                            sp[:, :, dia],
                                            sp[:, :, dia],
                                            masks_sb[:, None, :].to_broadcast(
                                                (P, 2, KB)
                                            ),
                                        )
                                    ee = smallp.tile([P, 2, QT], bf16, tag="ee")
                                    nc.scalar.activation(
                                        out=ee[:, :, q0:],
                                        in_=sp[:, :, q0:],
                                        func=mybir.ActivationFunctionType.Exp,
                                        scale=float(HS) ** -0.5,
                                    )
                                    he = (2 * hp) * (HS + 1)
                                    ho = (2 * hp + 1) * (HS + 1)
                                    nc.tensor.matmul(
                                        apair[:, q0:],
                                        v4e[:, kb, he : he + HS + 1],
                                        ee[:, 0, q0:],
                                        start=(kb == 0),
                                        stop=(kb == nkb - 1),
                                    )
                                    nc.tensor.matmul(
                                        apodd[:, q0:],
                                        v4e[:, kb, ho : ho + HS + 1],
                                        ee[:, 1, q0:],
                                        start=(kb == 0),
                                        stop=(kb == nkb - 1),
                                    )
                                # drain accumulators; stack both heads into one
                                # [128, QT] tile, batch the reciprocal of the
                                # two denominator rows, broadcast, normalize.
                                ua0 = denp.tile([HS, QT], f32, tag="ua0")
                                nc.vector.tensor_copy(ua0, apair[0:HS, :])
                                ua1 = denp.tile([HS, QT], f32, tag="ua1")
                                nc.vector.tensor_copy(ua1, apodd[0:HS, :])
                                dr0 = denp.tile([1, QT], f32, tag="dr0")
                                nc.vector.tensor_copy(dr0, apair[HS : HS + 1, :])
                                dr1 = denp.tile([1, QT], f32, tag="dr1")
                                nc.vector.tensor_copy(dr1, apodd[HS : HS + 1, :])
                                rc0 = denp.tile([1, QT], f32, tag="rc0")
                                nc.vector.reciprocal_approx_fast(out=rc0, in_=dr0)
                                rc1 = denp.tile([1, QT], f32, tag="rc1")
                                nc.vector.reciprocal_approx_fast(out=rc1, in_=dr1)
                                bc0 = denp.tile([HS, QT], f32, tag="bc0")
                                nc.gpsimd.partition_broadcast(bc0, rc0)
                                bc1 = denp.tile([HS, QT], f32, tag="bc1")
                                nc.gpsimd.partition_broadcast(bc1, rc1)
                                nc.vector.tensor_mul(attnT[0:HS, hp, qsl], ua0, bc0)
                                nc.vector.tensor_mul(
                                    attnT[HS : 2 * HS, hp, qsl], ua1, bc1
                                )
                            # proj for this qt's 4 token tiles, then RS chunk qt
                            for mtl in range(4):
                                mt = 4 * qt + mtl
                                prj = projp.tile([P, D], bf16, tag="prj")
                                for nh in range(D // QT):
                                    pps = pspr.tile([P, QT], f32, tag="pr")
                                    for ko in range(2):
                                        nc.tensor.matmul(
                                            pps,
                                            attnT[:, ko, mt * P : (mt + 1) * P],
                                            wp_sb[:, ko, nh * QT : (nh + 1) * QT],
                                            start=(ko == 0),
                                            stop=(ko == 1),
                                        )
                                    nc.vector.tensor_copy(
                                        prj[:, nh * QT : (nh + 1) * QT], pps
                                    )
                                nc.sync.dma_start(
                                    rs_in[qt].ap()[mtl * P : (mtl + 1) * P, :], prj
                                )
                            nc.gpsimd.collective_compute(
                                "ReduceScatter",
                                mybir.AluOpType.add,
                                replica_groups=REPLICA_GROUPS,
                                ins=[rs_in[qt].ap().opt()],
                                outs=[rs_out[qt].ap().opt()],
                            )
                            # LN1 for chunks whose RS landed ~a qt step ago
                            if qt >= 1:
                                emit_ln1(qt - 1, ln1p)
                        emit_ln1(NCK - 1, ln1p)
                        if DEBUG:
                            nc.sync.dma_start(dbg_q2T, q2T)
                            nc.sync.dma_start(dbg_k2T, k2T)
                            nc.sync.dma_start(dbg_v4e, v4e)
                            nc.sync.dma_start(dbg_attnT, attnT)
                            nc.sync.dma_start(dbg_x1r, x1r)
                            with tc.tile_pool(name="dbgp", bufs=1) as dbgp:
                                for st in range(NCK):
                                    dy = dbgp.tile([P, D], bf16, tag="dy", name=f"dy{st}")
                                    nc.gpsimd.dma_start(dy, rs_out[st].ap())
                                    nc.sync.dma_start(dbg_rsout[st], dy)

            # ------------- Phase B: FFN + LN2 -------------
            with tc.tile_pool(name="w1p", bufs=1) as w1p:
                # w1 DMA (scalar queue) overlaps the attention tail / last RS
                w1_sb = w1p.tile([P, D // P, F], bf16, tag="w1")
                nc.scalar.dma_start(w1_sb, w1.rearrange("(ko p) m -> p ko m", p=P))
                hT = w1p.tile([P, F // P, SL], bf16, tag="hT")
                x1T = w1p.tile([P, D // P, SL], bf16, tag="x1T")

                # LN2 constants (gpsimd queue, well ahead of use)
                g2b = w1p.tile([P, D], f32, tag="g2b")
                nc.gpsimd.dma_start(g2b, _bcast_row_ap(cvec.tensor, 3, D))
                be2b = w1p.tile([P, D], f32, tag="be2b")
                nc.gpsimd.dma_start(be2b, _bcast_row_ap(cvec.tensor, 4, D))
                b2b = w1p.tile([P, D], f32, tag="b2b")
                nc.gpsimd.dma_start(b2b, _bcast_row_ap(cvec.tensor, 5, D))

                with (
                    tc.tile_pool(name="ps_tr", bufs=2, space="PSUM") as pstr,
                    tc.tile_pool(name="ps_h", bufs=2, space="PSUM") as psh,
                ):
                    # transpose x1 into x1T (PE; cheap for bf16)
                    for st in range(SL // P):
                        for dk in range(D // P):
                            tp = pstr.tile([P, P], bf16, tag="tp")
                            nc.tensor.transpose(
                                tp, x1b[:, st, dk * P : (dk + 1) * P], identb
                            )
                            nc.scalar.copy(x1T[:, dk, st * P : (st + 1) * P], tp)

                    # FFN first matmul: hT[f, tok] = relu(w1.T @ x1T + b1)
                    for ft in range(F // P):
                        hps = psh.tile([P, SL], f32, tag="h")
                        for ko in range(D // P):
                            nc.tensor.matmul(
                                hps,
                                w1_sb[:, ko, ft * P : (ft + 1) * P],
                                x1T[:, ko, :],
                                start=(ko == 0),
                                stop=(ko == D // P - 1),
                            )
                        nc.scalar.activation(
                            out=hT[:, ft, :],
                            in_=hps,
                            func=mybir.ActivationFunctionType.Relu,
                            bias=b1_sb[:, ft : ft + 1],
                            scale=1.0,
                        )

                if DEBUG:
                    nc.sync.dma_start(dbg_hT, hT)

                # FFN second matmul: stream w2 tiles, accumulate [tok, d] in
                # 8 PSUM banks (4 token blocks x 2 d-halves)
                with (
                    tc.tile_pool(name="w2p", bufs=3) as w2p,
                    tc.tile_pool(name="ln2p", bufs=2) as ln2p,
                    tc.tile_pool(name="ps_y", bufs=8, space="PSUM") as psy,
                ):
                    NFB = F // P  # 32 w2 row-tiles
                    ypss = [
                        psy.tile([P, QT], f32, tag="yq", name=f"yq_{i}")
                        for i in range(8)
                    ]
                    for fb in range(NFB):
                        w2t = w2p.tile([P, D], bf16, tag="w2t")
                        nc.scalar.dma_start(w2t, w2[fb * P : (fb + 1) * P, :])
                        for mt in range(SL // P):
                            for dh in range(2):
                                nc.tensor.matmul(
                                    ypss[2 * mt + dh],
                                    hT[:, fb, mt * P : (mt + 1) * P],
                                    w2t[:, dh * QT : (dh + 1) * QT],
                                    start=(fb == 0),
                                    stop=(fb == NFB - 1),
                                )
                    # epilogue per token block: +residual +b2, LN2, out
                    for mt in range(SL // P):
                        z = ln2p.tile([P, D], f32, tag="z", name=f"z{mt}")
                        for dh in range(2):
                            dsl = slice(dh * QT, (dh + 1) * QT)
                            nc.vector.tensor_add(
                                z[:, dsl], ypss[2 * mt + dh], x1r[:, mt, dsl]
                            )
                        nc.vector.tensor_add(z, z, b2b)
                        stats = ln2p.tile([P, 2, 6], f32, tag="st2", name=f"s2{mt}")
                        zv = z.rearrange("p (s d) -> p s d", s=2)
                        nc.vector.bn_stats(out=stats[:, 0, :], in_=zv[:, 0, :])
                        nc.vector.bn_stats(out=stats[:, 1, :], in_=zv[:, 1, :])
                        mv = ln2p.tile([P, 2], f32, tag="mv2", name=f"m2{mt}")
                        nc.vector.bn_aggr(out=mv, in_=stats)
                        rstd = ln2p.tile([P, 1], f32, tag="rs2", name=f"r2{mt}")
                        nc.scalar.activation(
                            out=rstd,
                            in_=mv[:, 1:2],
                            func=mybir.ActivationFunctionType.Sqrt,
                            bias=eps_t,
                            scale=1.0,
                        )
                        nc.vector.reciprocal(rstd, rstd)
                        o = ln2p.tile([P, D], f32, tag="o", name=f"o{mt}")
                        nc.vector.tensor_scalar(
                            out=o,
                            in0=z,
                            scalar1=mv[:, 0:1],
                            scalar2=rstd,
                            op0=mybir.AluOpType.subtract,
                            op1=mybir.AluOpType.mult,
                        )
                        nc.vector.tensor_mul(o, o, g2b)
                        nc.vector.tensor_add(o, o, be2b)
                        nc.sync.dma_start(out[mt * P : (mt + 1) * P, :], o)

            ffn_keep_cm.__exit__(None, None, None)

    nc.compile()
    return nc


_NC_CACHE = []


def _get_nc():
    if not _NC_CACHE:
        _NC_CACHE.append(build_bass())
    return _NC_CACHE[0]


def _token_blocks(r):
    """Global token rows (within a batch element) owned by rank r, as NCK
    blocks of 128: block ck covers rows [512*ck + 128*r, 512*ck + 128*r + 128)."""
    return [slice(QT * ck + P * r, QT * ck + P * r + P) for ck in range(NCK)]


def make_in_maps(x, wq, wk, wv, w_proj, b_proj, w1, b1, w2, b2, g1, be1, g2, be2):
    x = np.asarray(x, dtype=np.float32)
    cat = lambda w, h0: np.ascontiguousarray(
        np.concatenate(
            [np.asarray(w[h0 + i], dtype=np.float32) for i in range(HPC)], axis=1
        ).astype(np_bf16)
    )
    cvec_rows = [b_proj, g1, be1, g2, be2, b2]
    cvec = np.ascontiguousarray(
        np.stack([np.asarray(v, dtype=np.float32) for v in cvec_rows])
    )
    w1c = np.ascontiguousarray(np.asarray(w1, dtype=np.float32).astype(np_bf16))
    w2c = np.ascontiguousarray(np.asarray(w2, dtype=np.float32).astype(np_bf16))
    b1c = np.ascontiguousarray(np.asarray(b1, dtype=np.float32))
    wpc = np.asarray(w_proj, dtype=np.float32)
    xTs = [np.ascontiguousarray(x[g].T.astype(np_bf16)) for g in range(B)]
    in_maps = []
    for c in range(NCORES):
        g, r = divmod(c, TP)
        h0 = HPC * r
        xs_blocks = np.concatenate([x[g, blk] for blk in _token_blocks(r)], axis=0)
        in_maps.append(
            {
                "xT": xTs[g],
                "xs": np.ascontiguousarray(xs_blocks),
                "wq2": cat(wq, h0),
                "wk2": cat(wk, h0),
                "wv4": cat(wv, h0),
                "wp": np.ascontiguousarray(
                    wpc[HPC * HS * r : HPC * HS * (r + 1)].astype(np_bf16)
                ),
                "w1": w1c,
                "w2": w2c,
                "cvec": cvec,
                "b1d": b1c,
            }
        )
    return in_maps


def assemble(results):
    full = np.empty((B, S, D), dtype=np.float32)
    for c in range(NCORES):
        g, r = divmod(c, TP)
        o = results[c]["out"]
        for ck, blk in enumerate(_token_blocks(r)):
            full[g, blk] = o[ck * P : (ck + 1) * P]
    return full


def kernel(**inputs):
    nc = _get_nc()
    in_maps = make_in_maps(**inputs)
    res = run_bass_kernel_spmd(nc, in_maps, core_ids=list(range(NCORES)))
    return assemble(res.results)
